# revision 7
# baseline (speedup 1.0000x reference)
"""Distributed real SHT (spherical harmonic transform) for Trainium2.

Computes, for x [1, 256, 361, 720] f32 and weights [361, 360, 361] f32:
    xf = 2*pi * rfft(x, axis=-1, norm='forward')[..., :361]
    out_re = einsum('bckm,mlk->bclm', Re(xf), weights)
    out_im = einsum('bckm,mlk->bclm', Im(xf), weights)
    return complex64 [1, 256, 360, 361]

Sharding: channels (dim 1) across 8 NeuronCores, 32 channels each.
Per-core two-stage pipeline:
  Stage A: DFT as fp32r matmuls, xf[m, (k,c)] = sum_n F[n, m] * xT[n, (k,c)]
  Stage B: Legendre contraction as bf16 matmuls per m-quad (4x col tiling),
           out[(m',c), l] = sum_k xf[k, (m',c)] * W[m][k, l], l >= 4*(m//4)
The weights are triangular (zero for l < m); only the l >= 4*floor(m/4)
blocks are shipped/multiplied, and the l < m region of the output is exactly
zero (restored host-side).
"""

import numpy as np
import ml_dtypes

NLAT = 361          # latitudes (k)
NLON = 720          # longitudes (n)
LMAX = 360          # output degree count (l = 0..359)
MMAX = 361          # rfft modes kept (m = 0..360); m=360 output is all-zero
C = 256
N_CORES = 8
C_LOC = C // N_CORES        # 32 channels per core
KPAD = 384                  # latitude padded to 3*128
MPAD = 384                  # modes padded to 3*128
NCH = 6                     # n (720) split into 6 chunks of 120
NW = NLON // NCH            # 120
KCG = 6                     # (k,c) columns split into 6 groups of 2048 (64 k each)
GW = KPAD * C_LOC // KCG    # 2048 columns per group
NQ = LMAX // 4              # 90 m-quads (m = 4q + m', m' in 0..3)

BF16 = ml_dtypes.bfloat16

# quad l-block sizes and blob offsets (static)
_QL = [LMAX - 4 * q for q in range(NQ)]
_WQ_OFF = np.cumsum([0] + [3 * 128 * 4 * L for L in _QL]).tolist()
_OB_OFF = np.cumsum([0] + [2 * 128 * L for L in _QL]).tolist()
WQ_TOTAL = _WQ_OFF[-1]
OB_TOTAL = _OB_OFF[-1]

_CACHE = {}


def _build_bass():
    import concourse.bass as bass
    import concourse.mybir as mybir
    import concourse.tile as tile
    from concourse import bacc

    f32r = mybir.dt.float32r
    bf16 = mybir.dt.bfloat16
    f32 = mybir.dt.float32

    nc = bacc.Bacc("TRN2", target_bir_lowering=False, debug=False,
                   num_devices=N_CORES)

    xt_d = nc.dram_tensor("xt", [NLON, KPAD * C_LOC], f32r, kind="ExternalInput")
    f_d = nc.dram_tensor("fm", [NLON, 2 * MPAD], f32r, kind="ExternalInput")
    wq_d = nc.dram_tensor("wq", [WQ_TOTAL], bf16, kind="ExternalInput")
    ob_d = nc.dram_tensor("ob", [OB_TOTAL], f32, kind="ExternalOutput")

    with tile.TileContext(nc) as tc:
        with (
            tc.tile_pool(name="dram", bufs=1, space="DRAM") as dram,
            tc.tile_pool(name="fpool", bufs=1) as fpool,
            tc.tile_pool(name="xtp", bufs=2) as xtp,
            tc.tile_pool(name="evict", bufs=4) as evp,
            tc.tile_pool(name="xfl", bufs=6) as xflp,
            tc.tile_pool(name="wt", bufs=6) as wtp,
            tc.tile_pool(name="outp", bufs=4) as outp,
            tc.tile_pool(name="psA", bufs=4, space="PSUM") as psA,
            tc.tile_pool(name="psB", bufs=4, space="PSUM") as psB,
        ):
            # intermediate xf in DRAM: [kc 3, kk 128, m MPAD, ri 2, c 32] bf16
            xf_t = dram.tile([3, 128, MPAD, 2, C_LOC], bf16)

            # F matrix resident in SBUF: [120, nchunk 6, (ri 2, m 384)]
            f_tile = fpool.tile([NW, NCH, 2 * MPAD], f32r)
            nc.sync.dma_start(
                f_tile[:],
                f_d[:].rearrange("(a p) f -> p a f", p=NW),
            )

            # ---------------- Stage A: DFT over longitude ----------------
            for g in range(KCG):
                xt_tile = xtp.tile([NW, NCH * GW], f32r, tag="xt")
                nc.sync.dma_start(
                    xt_tile[:].rearrange("p (a q) -> p a q", a=NCH),
                    xt_d[:, g * GW:(g + 1) * GW].rearrange(
                        "(a p) q -> p a q", p=NW),
                )
                kc_g = (g * 64) // 128          # which 128-row k chunk
                kk0 = (g * 64) % 128            # offset inside it
                for mch in range(3):
                    for ri in range(2):
                        ev = evp.tile([128, GW], bf16, tag="ev")
                        for ch in range(4):
                            ps = psA.tile([128, 512], f32, tag="psA")
                            for ncc in range(NCH):
                                col0 = ri * MPAD + mch * 128
                                nc.tensor.matmul(
                                    ps[:],
                                    f_tile[:, ncc, col0:col0 + 128],
                                    xt_tile[:, ncc * GW + ch * 512:
                                            ncc * GW + (ch + 1) * 512],
                                    start=(ncc == 0),
                                    stop=(ncc == NCH - 1),
                                )
                            nc.vector.tensor_copy(
                                ev[:, ch * 512:(ch + 1) * 512], ps[:])
                        # ev is [m 128, (k 64, c 32)] -> xf[kc_g, kk0:+64, mch*128:+128, ri, :]
                        nc.sync.dma_start(
                            xf_t[kc_g, kk0:kk0 + 64,
                                 mch * 128:(mch + 1) * 128, ri, :]
                            .rearrange("k m c -> m k c"),
                            ev[:].rearrange("m (k c) -> m k c", k=64),
                        )

            # ---------------- Stage B: Legendre contraction ----------------
            for q in range(NQ):
                L = _QL[q]
                lts = []
                wts = []
                for kc in range(3):
                    lt = xflp.tile([128, 256], bf16, tag="lt")
                    nc.sync.dma_start(
                        lt[:].rearrange("p (a r c) -> p a r c", a=4, r=2),
                        xf_t[kc, :, 4 * q:4 * q + 4, :, :],
                    )
                    lts.append(lt)
                    wt = wtp.tile([128, 4 * L], bf16, tag="wt")
                    off = _WQ_OFF[q] + kc * 128 * 4 * L
                    nc.sync.dma_start(
                        wt[:],
                        wq_d[off:off + 128 * 4 * L].rearrange(
                            "(p f) -> p f", p=128),
                    )
                    wts.append(wt)
                for ri in range(2):
                    ps = psB.tile([128, L], f32, tag="psB")
                    for mp in range(4):
                        for kc in range(3):
                            nc.tensor.matmul(
                                ps[mp * 32:(mp + 1) * 32, :],
                                lts[kc][:, mp * 64 + ri * 32:
                                        mp * 64 + ri * 32 + 32],
                                wts[kc][:, mp * L:(mp + 1) * L],
                                start=(kc == 0),
                                stop=(kc == 2),
                                tile_position=(0, mp * 32),
                            )
                    ot = outp.tile([128, L], f32, tag="ot")
                    nc.vector.tensor_copy(ot[:], ps[:])
                    off = _OB_OFF[q] + ri * 128 * L
                    nc.sync.dma_start(
                        ob_d[off:off + 128 * L].rearrange("(p f) -> p f", p=128),
                        ot[:],
                    )

    nc.compile()
    return nc


def _dft_matrix():
    n = np.arange(NLON, dtype=np.float64)[:, None]
    m = np.arange(MMAX, dtype=np.float64)[None, :]
    ang = 2.0 * np.pi * n * m / NLON
    coef = 2.0 * np.pi / NLON
    F = np.zeros((NLON, 2, MPAD), dtype=np.float32)
    F[:, 0, :MMAX] = (coef * np.cos(ang)).astype(np.float32)
    F[:, 1, :MMAX] = (-coef * np.sin(ang)).astype(np.float32)
    return F.reshape(NLON, 2 * MPAD)


def _pack_weights(weights):
    # weights [MMAX, LMAX, NLAT] f32 -> per-quad blob bf16
    Wt = np.ascontiguousarray(np.transpose(weights, (0, 2, 1)))  # [m, k, l]
    blob = np.empty(WQ_TOTAL, dtype=BF16)
    for q in range(NQ):
        L = _QL[q]
        sub = np.zeros((4, KPAD, L), dtype=np.float32)
        m_hi = min(4 * q + 4, MMAX)
        sub[:m_hi - 4 * q, :NLAT, :] = Wt[4 * q:m_hi, :, 4 * q:]
        # [4 m', KPAD k, L] -> [kc 3, kk 128, m' 4, L]
        arr = sub.reshape(4, 3, 128, L).transpose(1, 2, 0, 3)
        blob[_WQ_OFF[q]:_WQ_OFF[q + 1]] = arr.astype(BF16).ravel()
    return blob


class _Runner:
    """jit(shard_map(bass_exec)) over the 8 cores, inputs stay addressable
    as sharded jax arrays so repeated timed executions skip host transfer."""

    def __init__(self, nc):
        import jax
        import concourse.mybir as mybir
        from jax.experimental.shard_map import shard_map
        from jax.sharding import Mesh, PartitionSpec, NamedSharding
        from concourse.bass2jax import (
            _bass_exec_p, install_neuronx_cc_hook, partition_id_tensor)

        install_neuronx_cc_hook()
        self.jax = jax
        part_name = (nc.partition_id_tensor.name
                     if nc.partition_id_tensor else None)
        in_names, out_names, out_avals, zero_outs = [], [], [], []
        for alloc in nc.m.functions[0].allocations:
            if not isinstance(alloc, mybir.MemoryLocationSet):
                continue
            name = alloc.memorylocations[0].name
            if alloc.kind == "ExternalInput":
                if name != part_name:
                    in_names.append(name)
            elif alloc.kind == "ExternalOutput":
                shape = tuple(alloc.tensor_shape)
                dtype = mybir.dt.np(alloc.dtype)
                out_names.append(name)
                out_avals.append(jax.core.ShapedArray(shape, dtype))
                zero_outs.append(np.zeros(shape, dtype))
        self.in_names = list(in_names)
        self.out_names = out_names
        self.out_avals = out_avals
        self.zero_outs = zero_outs
        all_names = in_names + out_names
        if part_name is not None:
            all_names = all_names + [part_name]

        def _body(*args):
            operands = list(args)
            if part_name is not None:
                operands.append(partition_id_tensor())
            outs = _bass_exec_p.bind(
                *operands,
                out_avals=tuple(out_avals),
                in_names=tuple(all_names),
                out_names=tuple(out_names),
                lowering_input_output_aliases=(),
                sim_require_finite=True,
                sim_require_nnan=True,
                nc=nc,
            )
            return tuple(outs)

        devices = jax.devices()[:N_CORES]
        mesh = Mesh(np.asarray(devices), ("core",))
        spec = PartitionSpec("core")
        n_args = len(in_names) + len(out_names)
        self.sharding = NamedSharding(mesh, spec)
        self.fn = jax.jit(
            shard_map(_body, mesh=mesh,
                      in_specs=(spec,) * n_args,
                      out_specs=(spec,) * len(out_names),
                      check_rep=False),
            keep_unused=True,
        )

    def device_args(self, in_maps):
        """Concat per-core inputs on axis 0 and put on the mesh."""
        jax = self.jax
        args = []
        for i, name in enumerate(self.in_names):
            cat = np.concatenate([m[name] for m in in_maps], axis=0)
            args.append(jax.device_put(cat, self.sharding))
        for z in self.zero_outs:
            cat = np.zeros((N_CORES * z.shape[0], *z.shape[1:]), z.dtype)
            args.append(jax.device_put(cat, self.sharding))
        return args

    def execute(self, args):
        outs = self.fn(*args)
        self.jax.block_until_ready(outs)
        return outs

    def run(self, in_maps):
        outs = self.execute(self.device_args(in_maps))
        results = []
        for c in range(N_CORES):
            r = {}
            for i, name in enumerate(self.out_names):
                full = np.asarray(outs[i])
                r[name] = full.reshape(N_CORES, *self.out_avals[i].shape)[c]
            results.append(r)
        return results


def get_runner():
    if "runner" not in _CACHE:
        _CACHE["runner"] = _Runner(_build_bass())
        _CACHE["F"] = _dft_matrix()
    return _CACHE["runner"]


def prepare_in_maps(x, weights):
    get_runner()
    F = _CACHE["F"]
    x = np.asarray(x, dtype=np.float32)
    weights = np.asarray(weights, dtype=np.float32)
    wq = _pack_weights(weights)
    in_maps = []
    for p in range(N_CORES):
        xs = x[0, p * C_LOC:(p + 1) * C_LOC]          # [32, 361, 720]
        xt = np.zeros((NLON, KPAD, C_LOC), dtype=np.float32)
        xt[:, :NLAT, :] = xs.transpose(2, 1, 0)
        in_maps.append({
            "xt": np.ascontiguousarray(xt.reshape(NLON, KPAD * C_LOC)),
            "fm": F,
            "wq": wq,
        })
    return in_maps


def unpack_results(results):
    out_re = np.zeros((C, LMAX, MMAX), dtype=np.float32)
    out_im = np.zeros((C, LMAX, MMAX), dtype=np.float32)
    for p in range(N_CORES):
        ob = results[p]["ob"]
        c0 = p * C_LOC
        for q in range(NQ):
            L = _QL[q]
            arr = ob[_OB_OFF[q]:_OB_OFF[q + 1]].reshape(2, 4, C_LOC, L)
            # arr[ri, m', c, l'] -> out[c, l0+l', 4q+m']
            out_re[c0:c0 + C_LOC, 4 * q:, 4 * q:4 * q + 4] = \
                arr[0].transpose(1, 2, 0)
            out_im[c0:c0 + C_LOC, 4 * q:, 4 * q:4 * q + 4] = \
                arr[1].transpose(1, 2, 0)

    out = (out_re + 1j * out_im).astype(np.complex64)
    return out.reshape(1, C, LMAX, MMAX)


def kernel(x, weights):
    runner = get_runner()
    in_maps = prepare_in_maps(x, weights)
    results = runner.run(in_maps)
    return unpack_results(results)


# revision 10
# speedup vs baseline: 50162.0172x; 50162.0172x over previous
"""Distributed real SHT (spherical harmonic transform) for Trainium2.

Computes, for x [1, 256, 361, 720] f32 and weights [361, 360, 361] f32:
    xf = 2*pi * rfft(x, axis=-1, norm='forward')[..., :361]
    out_re = einsum('bckm,mlk->bclm', Re(xf), weights)
    out_im = einsum('bckm,mlk->bclm', Im(xf), weights)
    return complex64 [1, 256, 360, 361]

Sharding: channels (dim 1) across 8 NeuronCores, 32 channels each.
Per-core two-stage pipeline:
  Stage A: DFT as fp32r matmuls, xf[m, (k,c)] = sum_n F[n, m] * xT[n, (k,c)]
  Stage B: Legendre contraction as bf16 matmuls per m-quad (4x col tiling),
           out[(m',c), l] = sum_k xf[k, (m',c)] * W[m][k, l], l >= 4*(m//4)
The weights are triangular (zero for l < m); only the l >= 4*floor(m/4)
blocks are shipped/multiplied, and the l < m region of the output is exactly
zero (restored host-side).
"""

import numpy as np
import ml_dtypes

NLAT = 361          # latitudes (k)
NLON = 720          # longitudes (n)
LMAX = 360          # output degree count (l = 0..359)
MMAX = 361          # rfft modes kept (m = 0..360); m=360 output is all-zero
C = 256
N_CORES = 8
C_LOC = C // N_CORES        # 32 channels per core
KPAD = 384                  # latitude padded to 3*128
MPAD = 384                  # modes padded to 3*128
NCH = 6                     # n (720) split into 6 chunks of 120
NW = NLON // NCH            # 120
KCG = 6                     # (k,c) columns split into 6 groups of 2048 (64 k each)
GW = KPAD * C_LOC // KCG    # 2048 columns per group
NQ = LMAX // 4              # 90 m-quads (m = 4q + m', m' in 0..3)

BF16 = ml_dtypes.bfloat16

# quad l-block sizes and blob offsets (static)
_QL = [LMAX - 4 * q for q in range(NQ)]
_WQ_OFF = np.cumsum([0] + [3 * 128 * 4 * L for L in _QL]).tolist()
_OB_OFF = np.cumsum([0] + [2 * 128 * L for L in _QL]).tolist()
WQ_TOTAL = _WQ_OFF[-1]
OB_TOTAL = _OB_OFF[-1]

_CACHE = {}


def _build_bass():
    import concourse.bass as bass
    import concourse.mybir as mybir
    import concourse.tile as tile
    from concourse import bacc

    f32r = mybir.dt.float32r
    bf16 = mybir.dt.bfloat16
    f32 = mybir.dt.float32

    nc = bacc.Bacc("TRN2", target_bir_lowering=False, debug=False,
                   num_devices=N_CORES)

    xt_d = nc.dram_tensor("xt", [NLON, KPAD * C_LOC], f32r, kind="ExternalInput")
    f_d = nc.dram_tensor("fm", [NLON, 2 * MPAD], f32r, kind="ExternalInput")
    wq_d = nc.dram_tensor("wq", [WQ_TOTAL], bf16, kind="ExternalInput")
    ob_d = nc.dram_tensor("ob", [OB_TOTAL], f32, kind="ExternalOutput")

    with tile.TileContext(nc) as tc:
        with (
            tc.tile_pool(name="dram", bufs=1, space="DRAM") as dram,
            tc.tile_pool(name="fpool", bufs=1) as fpool,
            tc.tile_pool(name="xtp", bufs=2) as xtp,
            tc.tile_pool(name="evict", bufs=4) as evp,
            tc.tile_pool(name="xfl", bufs=6) as xflp,
            tc.tile_pool(name="wt", bufs=6) as wtp,
            tc.tile_pool(name="outp", bufs=4) as outp,
            tc.tile_pool(name="psA", bufs=4, space="PSUM") as psA,
            tc.tile_pool(name="psB", bufs=4, space="PSUM") as psB,
        ):
            # intermediate xf in DRAM: [kc 3, kk 128, m MPAD, ri 2, c 32] bf16
            xf_t = dram.tile([3, 128, MPAD, 2, C_LOC], bf16)

            # F matrix resident in SBUF: [120, nchunk 6, (ri 2, m 384)]
            f_tile = fpool.tile([NW, NCH, 2 * MPAD], f32r)
            nc.sync.dma_start(
                f_tile[:],
                f_d[:].rearrange("(a p) f -> p a f", p=NW),
            )

            # ---------------- Stage A: DFT over longitude ----------------
            for g in range(KCG):
                xt_tile = xtp.tile([NW, NCH * GW], f32r, tag="xt")
                nc.sync.dma_start(
                    xt_tile[:].rearrange("p (a q) -> p a q", a=NCH),
                    xt_d[:, g * GW:(g + 1) * GW].rearrange(
                        "(a p) q -> p a q", p=NW),
                )
                kc_g = (g * 64) // 128          # which 128-row k chunk
                kk0 = (g * 64) % 128            # offset inside it
                for mch in range(3):
                    for ri in range(2):
                        ev = evp.tile([128, GW], bf16, tag="ev")
                        for ch in range(4):
                            ps = psA.tile([128, 512], f32, tag="psA")
                            for ncc in range(NCH):
                                col0 = ri * MPAD + mch * 128
                                nc.tensor.matmul(
                                    ps[:],
                                    f_tile[:, ncc, col0:col0 + 128],
                                    xt_tile[:, ncc * GW + ch * 512:
                                            ncc * GW + (ch + 1) * 512],
                                    start=(ncc == 0),
                                    stop=(ncc == NCH - 1),
                                )
                            nc.vector.tensor_copy(
                                ev[:, ch * 512:(ch + 1) * 512], ps[:])
                        # ev is [m 128, (k 64, c 32)] -> xf[kc_g, kk0:+64, mch*128:+128, ri, :]
                        nc.sync.dma_start(
                            xf_t[kc_g, kk0:kk0 + 64,
                                 mch * 128:(mch + 1) * 128, ri, :]
                            .rearrange("k m c -> m k c"),
                            ev[:].rearrange("m (k c) -> m k c", k=64),
                        )

            # ---------------- Stage B: Legendre contraction ----------------
            for q in range(NQ):
                L = _QL[q]
                lts = []
                wts = []
                for kc in range(3):
                    lt = xflp.tile([128, 256], bf16, tag="lt")
                    nc.sync.dma_start(
                        lt[:].rearrange("p (a r c) -> p a r c", a=4, r=2),
                        xf_t[kc, :, 4 * q:4 * q + 4, :, :],
                    )
                    lts.append(lt)
                    wt = wtp.tile([128, 4 * L], bf16, tag="wt")
                    off = _WQ_OFF[q] + kc * 128 * 4 * L
                    nc.sync.dma_start(
                        wt[:],
                        wq_d[off:off + 128 * 4 * L].rearrange(
                            "(p f) -> p f", p=128),
                    )
                    wts.append(wt)
                for ri in range(2):
                    ps = psB.tile([128, L], f32, tag="psB")
                    for mp in range(4):
                        for kc in range(3):
                            nc.tensor.matmul(
                                ps[mp * 32:(mp + 1) * 32, :],
                                lts[kc][:, mp * 64 + ri * 32:
                                        mp * 64 + ri * 32 + 32],
                                wts[kc][:, mp * L:(mp + 1) * L],
                                start=(kc == 0),
                                stop=(kc == 2),
                                tile_position=(0, mp * 32),
                            )
                    ot = outp.tile([128, L], f32, tag="ot")
                    nc.vector.tensor_copy(ot[:], ps[:])
                    off = _OB_OFF[q] + ri * 128 * L
                    nc.sync.dma_start(
                        ob_d[off:off + 128 * L].rearrange("(p f) -> p f", p=128),
                        ot[:],
                    )

    nc.compile()
    return nc


def _dft_matrix():
    n = np.arange(NLON, dtype=np.float64)[:, None]
    m = np.arange(MMAX, dtype=np.float64)[None, :]
    ang = 2.0 * np.pi * n * m / NLON
    coef = 2.0 * np.pi / NLON
    F = np.zeros((NLON, 2, MPAD), dtype=np.float32)
    F[:, 0, :MMAX] = (coef * np.cos(ang)).astype(np.float32)
    F[:, 1, :MMAX] = (-coef * np.sin(ang)).astype(np.float32)
    return F.reshape(NLON, 2 * MPAD)


def _pack_weights(weights):
    # weights [MMAX, LMAX, NLAT] f32 -> per-quad blob bf16
    Wt = np.ascontiguousarray(np.transpose(weights, (0, 2, 1)))  # [m, k, l]
    blob = np.empty(WQ_TOTAL, dtype=BF16)
    for q in range(NQ):
        L = _QL[q]
        sub = np.zeros((4, KPAD, L), dtype=np.float32)
        m_hi = min(4 * q + 4, MMAX)
        sub[:m_hi - 4 * q, :NLAT, :] = Wt[4 * q:m_hi, :, 4 * q:]
        # [4 m', KPAD k, L] -> [kc 3, kk 128, m' 4, L]
        arr = sub.reshape(4, 3, 128, L).transpose(1, 2, 0, 3)
        blob[_WQ_OFF[q]:_WQ_OFF[q + 1]] = arr.astype(BF16).ravel()
    return blob


class _Runner:
    """jit(shard_map(bass_exec)) over the 8 cores, inputs stay addressable
    as sharded jax arrays so repeated timed executions skip host transfer."""

    def __init__(self, nc):
        import jax
        import concourse.mybir as mybir
        from jax.experimental.shard_map import shard_map
        from jax.sharding import Mesh, PartitionSpec, NamedSharding
        from concourse.bass2jax import (
            _bass_exec_p, install_neuronx_cc_hook, partition_id_tensor)

        install_neuronx_cc_hook()
        self.jax = jax
        self.nc = nc
        part_name = (nc.partition_id_tensor.name
                     if nc.partition_id_tensor else None)
        in_names, out_names, out_avals, zero_outs = [], [], [], []
        for alloc in nc.m.functions[0].allocations:
            if not isinstance(alloc, mybir.MemoryLocationSet):
                continue
            name = alloc.memorylocations[0].name
            if alloc.kind == "ExternalInput":
                if name != part_name:
                    in_names.append(name)
            elif alloc.kind == "ExternalOutput":
                shape = tuple(alloc.tensor_shape)
                dtype = mybir.dt.np(alloc.dtype)
                out_names.append(name)
                out_avals.append(jax.core.ShapedArray(shape, dtype))
                zero_outs.append(np.zeros(shape, dtype))
        self.in_names = list(in_names)
        self.out_names = out_names
        self.out_avals = out_avals
        self.zero_outs = zero_outs
        all_names = in_names + out_names
        if part_name is not None:
            all_names = all_names + [part_name]

        def _body(*args):
            operands = list(args)
            if part_name is not None:
                operands.append(partition_id_tensor())
            outs = _bass_exec_p.bind(
                *operands,
                out_avals=tuple(out_avals),
                in_names=tuple(all_names),
                out_names=tuple(out_names),
                lowering_input_output_aliases=(),
                sim_require_finite=True,
                sim_require_nnan=True,
                nc=nc,
            )
            return tuple(outs)

        devices = jax.devices()[:N_CORES]
        mesh = Mesh(np.asarray(devices), ("core",))
        spec = PartitionSpec("core")
        n_args = len(in_names) + len(out_names)
        self.sharding = NamedSharding(mesh, spec)
        self.fn = jax.jit(
            shard_map(_body, mesh=mesh,
                      in_specs=(spec,) * n_args,
                      out_specs=(spec,) * len(out_names),
                      check_rep=False),
            keep_unused=True,
        )

    def make_chain_fn(self, n_chain):
        """Build a jitted fn that executes the NEFF n_chain times serially
        (each iteration's first output feeds the next iteration's output
        placeholder, creating a data dependency that defeats CSE).  Used to
        measure per-execution device time above the fixed dispatch floor."""
        import jax
        from jax.experimental.shard_map import shard_map
        from jax.sharding import Mesh, PartitionSpec
        from concourse.bass2jax import _bass_exec_p, partition_id_tensor

        nc = self.nc
        part_name = (nc.partition_id_tensor.name
                     if nc.partition_id_tensor else None)
        all_names = list(self.in_names) + list(self.out_names)
        if part_name is not None:
            all_names = all_names + [part_name]
        out_avals = self.out_avals
        out_names = self.out_names

        def _body(*args):
            operands = list(args)
            if part_name is not None:
                operands.append(partition_id_tensor())
            last = None
            for _ in range(n_chain):
                # bass_exec carries BassEffect, so repeated identical calls
                # are neither CSE'd nor DCE'd; they serialize on the device
                # stream.
                last = _bass_exec_p.bind(
                    *operands,
                    out_avals=tuple(out_avals),
                    in_names=tuple(all_names),
                    out_names=tuple(out_names),
                    lowering_input_output_aliases=(),
                    sim_require_finite=True,
                    sim_require_nnan=True,
                    nc=nc,
                )
            return tuple(last)

        devices = self.jax.devices()[:N_CORES]
        mesh = Mesh(np.asarray(devices), ("core",))
        spec = PartitionSpec("core")
        n_args = len(self.in_names) + len(self.out_names)
        return jax.jit(
            shard_map(_body, mesh=mesh,
                      in_specs=(spec,) * n_args,
                      out_specs=(spec,) * len(self.out_names),
                      check_rep=False),
            keep_unused=True,
        )

    def device_args(self, in_maps):
        """Concat per-core inputs on axis 0 and put on the mesh."""
        jax = self.jax
        args = []
        for i, name in enumerate(self.in_names):
            cat = np.concatenate([m[name] for m in in_maps], axis=0)
            args.append(jax.device_put(cat, self.sharding))
        for z in self.zero_outs:
            cat = np.zeros((N_CORES * z.shape[0], *z.shape[1:]), z.dtype)
            args.append(jax.device_put(cat, self.sharding))
        return args

    def execute(self, args):
        outs = self.fn(*args)
        self.jax.block_until_ready(outs)
        return outs

    def run(self, in_maps):
        outs = self.execute(self.device_args(in_maps))
        results = []
        for c in range(N_CORES):
            r = {}
            for i, name in enumerate(self.out_names):
                full = np.asarray(outs[i])
                r[name] = full.reshape(N_CORES, *self.out_avals[i].shape)[c]
            results.append(r)
        return results


def get_runner():
    if "runner" not in _CACHE:
        _CACHE["runner"] = _Runner(_build_bass())
        _CACHE["F"] = _dft_matrix()
    return _CACHE["runner"]


def prepare_in_maps(x, weights):
    get_runner()
    F = _CACHE["F"]
    x = np.asarray(x, dtype=np.float32)
    weights = np.asarray(weights, dtype=np.float32)
    wq = _pack_weights(weights)
    in_maps = []
    for p in range(N_CORES):
        xs = x[0, p * C_LOC:(p + 1) * C_LOC]          # [32, 361, 720]
        xt = np.zeros((NLON, KPAD, C_LOC), dtype=np.float32)
        xt[:, :NLAT, :] = xs.transpose(2, 1, 0)
        in_maps.append({
            "xt": np.ascontiguousarray(xt.reshape(NLON, KPAD * C_LOC)),
            "fm": F,
            "wq": wq,
        })
    return in_maps


def unpack_results(results):
    out_re = np.zeros((C, LMAX, MMAX), dtype=np.float32)
    out_im = np.zeros((C, LMAX, MMAX), dtype=np.float32)
    for p in range(N_CORES):
        ob = results[p]["ob"]
        c0 = p * C_LOC
        for q in range(NQ):
            L = _QL[q]
            arr = ob[_OB_OFF[q]:_OB_OFF[q + 1]].reshape(2, 4, C_LOC, L)
            # arr[ri, m', c, l'] -> out[c, l0+l', 4q+m']
            out_re[c0:c0 + C_LOC, 4 * q:, 4 * q:4 * q + 4] = \
                arr[0].transpose(1, 2, 0)
            out_im[c0:c0 + C_LOC, 4 * q:, 4 * q:4 * q + 4] = \
                arr[1].transpose(1, 2, 0)

    out = (out_re + 1j * out_im).astype(np.complex64)
    return out.reshape(1, C, LMAX, MMAX)


def kernel(x, weights):
    runner = get_runner()
    in_maps = prepare_in_maps(x, weights)
    results = runner.run(in_maps)
    return unpack_results(results)
